# revision 1
# baseline (speedup 1.0000x reference)
"""GPT-NeoX attention (B=4, S=1024, D=2048, H=16) on 8 TRN2 NeuronCores.

Tensor-parallel over heads: 2 heads per core. Each core computes its slice
of the fused QKV projection, RoPE, causal attention, and writes the
transposed per-head output [hd, S]; the host concatenates heads.

All on-chip matmuls use float32r (full PE speed at free-dim>=256) with
fp32 PSUM accumulation. Layouts are chosen so no on-chip transposes are
needed:
  - x is fed transposed  xT[feature, token]
  - q,k are produced transposed  qT/kT[hd, token]  (RoPE applied in place)
  - v is produced natural  v[token, hd]  via a second projection pass
  - scores are computed transposed  sT[k_token, q_token]
  - out is produced transposed  oT[hd, q_token] = v.T @ expT
  - softmax sum over k = ones-vector matmul; normalization applied to oT
    via a K=1 broadcast matmul of the reciprocal row.
"""

import os

import numpy as np

import concourse.bass as bass
import concourse.tile as tile
from concourse import bacc, mybir

# Problem constants (contract: nn_GPTNeoXAttention, fixed shapes)
B, S, D = 4, 1024, 2048
H = 16
HD = 128  # head dim
NCORES = 8
HPC = H // NCORES  # heads per core
ROPE_BASE = 10000.0
T = B * S  # 4096 tokens
KC = D // 128  # 16 contraction chunks of the model dim
NSL = 512  # token-slice width for the qk projection
NHALF = S // NSL  # 2 slices per batch
QCH = S // 512  # q slices per sequence in attention
SCALE = 1.0 / float(np.sqrt(HD))

F32 = mybir.dt.float32
F32R = mybir.dt.float32r

_CACHE = {}


def _build_program():
    nc = bacc.Bacc(
        "TRN2", target_bir_lowering=False, debug=False, num_devices=NCORES
    )

    x_d = nc.dram_tensor("x", [128, KC, T], F32R, kind="ExternalInput")
    w_d = nc.dram_tensor("w", [128, KC, 6 * HD], F32R, kind="ExternalInput")
    bqk_d = nc.dram_tensor("bqk", [128, 4], F32, kind="ExternalInput")
    bv_d = nc.dram_tensor("bv", [128, 2 * HD], F32, kind="ExternalInput")
    cos_d = nc.dram_tensor("cosT", [128, S], F32, kind="ExternalInput")
    sin_d = nc.dram_tensor("sinS", [128, S], F32, kind="ExternalInput")
    mask_d = nc.dram_tensor("masks", [128, 4, 512], F32, kind="ExternalInput")
    rot_d = nc.dram_tensor("rotT", [128, 128], F32R, kind="ExternalInput")
    ones_d = nc.dram_tensor("ones", [128, 128], F32R, kind="ExternalInput")
    out_d = nc.dram_tensor("out", [HPC, HD, B, S], F32, kind="ExternalOutput")

    x_ap = x_d.ap()
    w_ap = w_d.ap()
    out_ap = out_d.ap()

    Exp = mybir.ActivationFunctionType.Exp
    Identity = mybir.ActivationFunctionType.Identity

    with tile.TileContext(nc) as tc:
        with (
            tc.tile_pool(name="singles", bufs=1) as singles,
            tc.tile_pool(name="xin", bufs=2) as xin_pool,
            tc.tile_pool(name="qk", bufs=6) as qk_pool,
            tc.tile_pool(name="vp", bufs=2) as v_pool,
            tc.tile_pool(name="expp", bufs=4) as exp_pool,
            tc.tile_pool(name="tmp", bufs=3) as tmp_pool,
            tc.tile_pool(name="outp", bufs=3) as out_pool,
            tc.tile_pool(name="rcp", bufs=2) as rcp_pool,
            tc.tile_pool(name="ps_mix", bufs=2, space="PSUM") as ps_mix,
            tc.tile_pool(name="ps_s", bufs=2, space="PSUM") as ps_s,
            tc.tile_pool(name="ps_o", bufs=2, space="PSUM") as ps_o,
            tc.tile_pool(name="ps_sum", bufs=2, space="PSUM") as ps_sum,
        ):
            w_sb = singles.tile([128, KC, 6 * HD], F32R)
            for kc in range(KC):
                nc.scalar.dma_start(out=w_sb[:, kc, :], in_=w_ap[:, kc, :])
            cos_sb = singles.tile([128, S], F32)
            nc.gpsimd.dma_start(out=cos_sb, in_=cos_d.ap())
            sin_sb = singles.tile([128, S], F32)
            nc.gpsimd.dma_start(out=sin_sb, in_=sin_d.ap())
            bqk_sb = singles.tile([128, 4], F32)
            nc.gpsimd.dma_start(out=bqk_sb, in_=bqk_d.ap())
            bv_sb = singles.tile([128, 2 * HD], F32)
            nc.gpsimd.dma_start(out=bv_sb, in_=bv_d.ap())
            mask_sb = singles.tile([128, 4, 512], F32)
            nc.gpsimd.dma_start(out=mask_sb, in_=mask_d.ap())
            rot_sb = singles.tile([128, 128], F32R)
            nc.gpsimd.dma_start(out=rot_sb, in_=rot_d.ap())
            # ones[128,128] lhsT: ones.T @ expT = sum over k, replicated
            # across all 128 output partitions (broadcast-ready layout)
            ones_sb = singles.tile([128, 128], F32R)
            nc.gpsimd.dma_start(out=ones_sb, in_=ones_d.ap())

            for b in range(B):
                # feature-major q/k tiles for this batch:
                # m=0: q head0, m=1: q head1, m=2: k head0, m=3: k head1
                qk_tiles = [
                    qk_pool.tile([128, S], F32R, tag="qkt", name=f"qkt_{b}_{i}")
                    for i in range(4)
                ]
                # natural-layout v for this batch: [token(128), chunk, 2*HD]
                v_sb = v_pool.tile([128, S // 128, 2 * HD], F32R)

                for half in range(NHALF):
                    t0 = b * S + half * NSL  # global token offset
                    xsb = xin_pool.tile([128, KC, NSL], F32R)
                    nc.sync.dma_start(out=xsb, in_=x_ap[:, :, t0 : t0 + NSL])

                    sl = slice(half * NSL, (half + 1) * NSL)
                    # ---- q/k projection (transposed out: [feature, token]) ----
                    for m in range(4):
                        ps = ps_mix.tile([128, NSL], F32, tag="ps")
                        for kc in range(KC):
                            nc.tensor.matmul(
                                ps,
                                w_sb[:, kc, m * 128 : (m + 1) * 128],
                                xsb[:, kc, :],
                                start=(kc == 0),
                                stop=(kc == KC - 1),
                            )
                        # bias add (per-partition scalar) on ACT, PSUM -> SBUF
                        qb = tmp_pool.tile([128, NSL], F32R, tag="qb")
                        nc.scalar.activation(
                            qb, ps, Identity, bias=bqk_sb[:, m : m + 1], scale=1.0
                        )
                        # RoPE: rotate_half via PE permutation matmul, then
                        # same-partition elementwise combine on DVE.
                        dst = qk_tiles[m][:, sl]
                        ps2 = ps_mix.tile([128, NSL], F32, tag="ps")
                        nc.tensor.matmul(
                            ps2,
                            rot_sb,
                            qb,
                            start=True,
                            stop=True,
                        )
                        tmp2 = tmp_pool.tile([128, NSL], F32, tag="tmp2")
                        nc.vector.tensor_mul(tmp2, ps2, sin_sb[:, sl])
                        nc.vector.tensor_mul(dst, qb, cos_sb[:, sl])
                        nc.vector.tensor_add(dst, dst, tmp2)

                    # ---- v projection (natural out: [token, feature]) ----
                    for t in range(NSL // 128):
                        psv = ps_mix.tile([128, 2 * HD], F32, tag="ps")
                        for kc in range(KC):
                            nc.tensor.matmul(
                                psv,
                                xsb[:, kc, t * 128 : (t + 1) * 128],
                                w_sb[:, kc, 4 * 128 : 6 * 128],
                                start=(kc == 0),
                                stop=(kc == KC - 1),
                            )
                        nc.vector.tensor_add(
                            v_sb[:, half * (NSL // 128) + t, :], psv, bv_sb
                        )

                # ---- attention for this batch ----
                for h in range(HPC):
                    qT = qk_tiles[h]
                    kT = qk_tiles[2 + h]
                    for qs in range(QCH):
                        nk = (qs * 512 + 512) // 128  # causal: k chunks needed
                        ps_out = ps_o.tile([128, 512], F32)
                        ps_sm = ps_sum.tile([128, 512], F32)
                        qsl = slice(qs * 512, (qs + 1) * 512)
                        for ki in range(nk):
                            pss = ps_s.tile([128, 512], F32, tag="sc")
                            nc.tensor.matmul(
                                pss,
                                kT[:, ki * 128 : (ki + 1) * 128],
                                qT[:, qsl],
                                start=True,
                                stop=True,
                            )
                            e = exp_pool.tile([128, 512], F32R, tag="e")
                            nc.scalar.activation(e, pss, Exp, scale=SCALE)
                            off = ki * 128 - qs * 512
                            if 0 <= off <= 384:
                                nc.vector.tensor_mul(
                                    e, e, mask_sb[:, off // 128, :]
                                )
                            nc.tensor.matmul(
                                ps_out,
                                v_sb[:, ki, h * HD : (h + 1) * HD],
                                e,
                                start=(ki == 0),
                                stop=(ki == nk - 1),
                            )
                            nc.tensor.matmul(
                                ps_sm,
                                ones_sb,
                                e,
                                start=(ki == 0),
                                stop=(ki == nk - 1),
                            )
                        rc = rcp_pool.tile([128, 512], F32)
                        nc.vector.reciprocal_approx_fast(out=rc, in_=ps_sm)
                        o = out_pool.tile([128, 512], F32)
                        nc.vector.tensor_mul(o, ps_out, rc)
                        nc.sync.dma_start(
                            out=out_ap[h, :, b, qsl], in_=o
                        )

    nc.compile()
    return nc


def _prep_shared(hidden_states):
    x2 = np.ascontiguousarray(hidden_states.reshape(T, D).T)  # [D, T]
    x_host = np.ascontiguousarray(
        x2.reshape(KC, 128, T).transpose(1, 0, 2)
    )  # [128, KC, T]

    inv = 1.0 / (ROPE_BASE ** (np.arange(0, HD, 2, dtype=np.float64) / HD))
    f = np.outer(inv, np.arange(S, dtype=np.float64))  # [64, S]
    cosT = np.concatenate([np.cos(f), np.cos(f)], axis=0).astype(np.float32)
    sinS = np.concatenate([np.sin(f), np.sin(f)], axis=0).astype(np.float32)

    p = np.arange(128)[:, None]
    fcol = np.arange(512)[None, :]
    masks = np.stack(
        [(fcol >= p + o).astype(np.float32) for o in (0, 128, 256, 384)], axis=1
    )  # [128, 4, 512]
    masks = np.ascontiguousarray(masks)

    # rotate_half as a matmul: out = lhsT.T @ rhs with lhsT = rotT gives
    # (R @ q)[i] = -q[i+64] (i<64), q[i-64] (i>=64)
    rotT = np.zeros((128, 128), np.float32)
    rotT[np.arange(64), np.arange(64) + 64] = 1.0
    rotT[np.arange(64) + 64, np.arange(64)] = -1.0
    return x_host, cosT, sinS, masks, rotT


def _core_rows(c):
    h0, h1 = 2 * c, 2 * c + 1
    rows = []
    for part in range(3):  # q, k, v blocks
        for h in (h0, h1):
            base = h * 3 * HD + part * HD
            rows.extend(range(base, base + HD))
    return np.asarray(rows)


def _prep_core(w_qkv, b_qkv, c):
    rows = _core_rows(c)
    wT = np.ascontiguousarray(w_qkv[rows, :].T)  # [D, 768]
    w_host = np.ascontiguousarray(
        wT.reshape(KC, 128, 6 * HD).transpose(1, 0, 2)
    )  # [128, KC, 768]
    b_sel = b_qkv[rows]
    bqk = np.ascontiguousarray(b_sel[: 4 * 128].reshape(4, 128).T)  # [128, 4]
    bv = np.ascontiguousarray(
        np.broadcast_to(b_sel[4 * 128 :], (128, 2 * HD))
    )  # [128, 256]
    return w_host, bqk, bv


def _make_in_maps(hidden_states, w_qkv, b_qkv):
    x_host, cosT, sinS, masks, rotT = _prep_shared(hidden_states)
    in_maps = []
    for c in range(NCORES):
        w_host, bqk, bv = _prep_core(w_qkv, b_qkv, c)
        in_maps.append(
            {
                "x": x_host,
                "w": w_host,
                "bqk": bqk,
                "bv": bv,
                "cosT": cosT,
                "sinS": sinS,
                "masks": masks,
                "rotT": rotT,
                "ones": np.ones((128, 128), np.float32),
            }
        )
    return in_maps


def _assemble(results):
    outs = np.stack([results[c]["out"] for c in range(NCORES)])
    # [NCORES, HPC, HD, B, S] -> [B, S, H*HD]
    return np.ascontiguousarray(
        outs.reshape(H, HD, B, S).transpose(2, 3, 0, 1).reshape(B, S, D)
    )


def run(hidden_states, w_qkv, b_qkv, trace=False):
    from concourse.bass_utils import run_bass_kernel_spmd

    if "nc" not in _CACHE:
        _CACHE["nc"] = _build_program()
    nc = _CACHE["nc"]
    in_maps = _make_in_maps(
        np.asarray(hidden_states, dtype=np.float32),
        np.asarray(w_qkv, dtype=np.float32),
        np.asarray(b_qkv, dtype=np.float32),
    )
    res = run_bass_kernel_spmd(
        nc, in_maps, core_ids=list(range(NCORES)), trace=trace
    )
    out = _assemble(res.results)
    return out, res


def kernel(hidden_states, w_qkv, b_qkv):
    trace = os.environ.get("KERNEL_TRACE", "0") == "1"
    out, _res = run(hidden_states, w_qkv, b_qkv, trace=trace)
    return out



# revision 2
# speedup vs baseline: 1.3508x; 1.3508x over previous
"""GPT-NeoX attention (B=4, S=1024, D=2048, H=16) on 8 TRN2 NeuronCores.

Tensor-parallel over heads: 2 heads per core. Each core computes its slice
of the fused QKV projection, RoPE, causal attention, and writes the
transposed per-head output [hd, S] in bf16; the host concatenates heads.

All on-chip matmuls use bf16 operands (1 cycle/row at any free size) with
fp32 PSUM accumulation. Attention computes only the valid causal region:
  - scores sT[k_chunk, q] for q >= k_chunk_start only
  - AV / ones-denominator accumulate per 512-wide q half with per-k-chunk
    clipped q ranges (element-wise PSUM accumulation handles the ragged
    start offsets)
Scores+exp for batch b are software-pipelined into the projection matmul
chains of the next segment so the scalar-engine exp latency never stalls
the in-order PE queue.
"""

import os

import numpy as np
import ml_dtypes

import concourse.bass as bass
import concourse.tile as tile
from concourse import bacc, mybir

# Problem constants (contract: nn_GPTNeoXAttention, fixed shapes)
B, S, D = 4, 1024, 2048
H = 16
HD = 128  # head dim
NCORES = 8
HPC = H // NCORES  # heads per core
ROPE_BASE = 10000.0
T = B * S  # 4096 tokens
KC = D // 128  # 16 contraction chunks of the model dim
NSL = 512  # token-slice width per projection segment
NSEG = B * 2
SCALE = 1.0 / float(np.sqrt(HD))

F32 = mybir.dt.float32
BF16 = mybir.dt.bfloat16
BF = ml_dtypes.bfloat16

_CACHE = {}


def _build_program():
    nc = bacc.Bacc(
        "TRN2", target_bir_lowering=False, debug=False, num_devices=NCORES
    )

    x_d = nc.dram_tensor("x", [128, NSEG, KC, NSL], BF16, kind="ExternalInput")
    w_d = nc.dram_tensor("w", [128, KC, 6 * HD], BF16, kind="ExternalInput")
    bqk_d = nc.dram_tensor("bqk", [128, 4], F32, kind="ExternalInput")
    bv_d = nc.dram_tensor("bv", [128, 2 * HD], F32, kind="ExternalInput")
    cos_d = nc.dram_tensor("cosT", [128, S], F32, kind="ExternalInput")
    sin_d = nc.dram_tensor("sinS", [128, S], F32, kind="ExternalInput")
    mask_d = nc.dram_tensor("mask0", [128, 128], F32, kind="ExternalInput")
    rot_d = nc.dram_tensor("rotT", [128, 128], BF16, kind="ExternalInput")
    ones_d = nc.dram_tensor("ones", [128, 128], BF16, kind="ExternalInput")
    out_d = nc.dram_tensor("out", [HPC, HD, B, S], BF16, kind="ExternalOutput")

    x_ap = x_d.ap()
    w_ap = w_d.ap()
    out_ap = out_d.ap()

    Exp = mybir.ActivationFunctionType.Exp
    Identity = mybir.ActivationFunctionType.Identity

    with tile.TileContext(nc) as tc:
        with (
            tc.tile_pool(name="singles", bufs=1) as singles,
            tc.tile_pool(name="xin", bufs=2) as xin_pool,
            tc.tile_pool(name="qk", bufs=8) as qk_pool,
            tc.tile_pool(name="vp", bufs=2) as v_pool,
            tc.tile_pool(name="ep", bufs=2) as e_pool,
            tc.tile_pool(name="qbp", bufs=3) as qb_pool,
            tc.tile_pool(name="tmp", bufs=3) as tmp_pool,
            tc.tile_pool(name="outp", bufs=3) as out_pool,
            tc.tile_pool(name="rcp", bufs=2) as rcp_pool,
            tc.tile_pool(name="ps_proj", bufs=2, space="PSUM") as ps_proj,
            tc.tile_pool(name="ps_misc", bufs=3, space="PSUM") as ps_misc,
            tc.tile_pool(name="ps_o", bufs=2, space="PSUM") as ps_o_pool,
            tc.tile_pool(name="ps_d", bufs=1, space="PSUM") as ps_d_pool,
        ):
            w_sb = singles.tile([128, KC, 6 * HD], BF16)
            for kc in range(KC):
                nc.scalar.dma_start(out=w_sb[:, kc, :], in_=w_ap[:, kc, :])
            cos_sb = singles.tile([128, S], F32)
            nc.gpsimd.dma_start(out=cos_sb, in_=cos_d.ap())
            sin_sb = singles.tile([128, S], F32)
            nc.gpsimd.dma_start(out=sin_sb, in_=sin_d.ap())
            bqk_sb = singles.tile([128, 4], F32)
            nc.gpsimd.dma_start(out=bqk_sb, in_=bqk_d.ap())
            bv_sb = singles.tile([128, 2 * HD], F32)
            nc.gpsimd.dma_start(out=bv_sb, in_=bv_d.ap())
            mask_sb = singles.tile([128, 128], F32)
            nc.gpsimd.dma_start(out=mask_sb, in_=mask_d.ap())
            rot_sb = singles.tile([128, 128], BF16)
            nc.gpsimd.dma_start(out=rot_sb, in_=rot_d.ap())
            ones_sb = singles.tile([128, 128], BF16)
            nc.gpsimd.dma_start(out=ones_sb, in_=ones_d.ap())

            qk_tiles = {}  # b -> [qh0, qh1, kh0, kh1] feature-major tiles
            v_tiles = {}  # b -> natural-layout v tile
            e_tiles = {}  # (b, h) -> exp'd scores, k-chunk-major

            def load_x(seg):
                xsb = xin_pool.tile(
                    [128, KC, NSL], BF16, tag="x", name=f"xsb_{seg}"
                )
                for c in range(4):
                    nc.sync.dma_start(
                        out=xsb[:, 4 * c : 4 * c + 4, :],
                        in_=x_ap[:, seg, 4 * c : 4 * c + 4, :],
                    )
                return xsb

            def proj(seg, xsb, fillers):
                """QKV projection for one 512-token segment; fillers are
                emitted between matmul chains to keep other engines fed."""
                b, half = divmod(seg, 2)
                if half == 0:
                    qk_tiles[b] = [
                        qk_pool.tile(
                            [128, S], BF16, tag="qkt", name=f"qkt_{b}_{i}"
                        )
                        for i in range(4)
                    ]
                    v_tiles[b] = v_pool.tile(
                        [128, S // 128, 2 * HD], BF16, tag="v", name=f"v_{b}"
                    )
                fi = 0
                sl = slice(half * NSL, (half + 1) * NSL)
                for m in range(4):
                    ps = ps_proj.tile(
                        [128, NSL], F32, tag="ps", name=f"psqk_{seg}_{m}"
                    )
                    for kc in range(KC):
                        nc.tensor.matmul(
                            ps,
                            w_sb[:, kc, m * 128 : (m + 1) * 128],
                            xsb[:, kc, :],
                            start=(kc == 0),
                            stop=(kc == KC - 1),
                        )
                    qb = qb_pool.tile(
                        [128, NSL], BF16, tag="qb", name=f"qb_{seg}_{m}"
                    )
                    nc.scalar.activation(
                        qb, ps, Identity, bias=bqk_sb[:, m : m + 1], scale=1.0
                    )
                    ps2 = ps_misc.tile(
                        [128, NSL], F32, tag="m", name=f"psrot_{seg}_{m}"
                    )
                    nc.tensor.matmul(ps2, rot_sb, qb, start=True, stop=True)
                    dst = qk_tiles[b][m][:, sl]
                    tmp2 = tmp_pool.tile(
                        [128, NSL], BF16, tag="t", name=f"tmp_{seg}_{m}"
                    )
                    nc.vector.tensor_mul(tmp2, ps2, sin_sb[:, sl])
                    nc.vector.tensor_mul(dst, qb, cos_sb[:, sl])
                    nc.vector.tensor_add(dst, dst, tmp2)
                    if fi < len(fillers):
                        fillers[fi]()
                        fi += 1
                for t in range(NSL // 128):
                    psv = ps_proj.tile(
                        [128, 2 * HD],
                        F32,
                        tag="ps",
                        padded_shape=[128, NSL],
                        name=f"psv_{seg}_{t}",
                    )
                    for kc in range(KC):
                        nc.tensor.matmul(
                            psv,
                            xsb[:, kc, t * 128 : (t + 1) * 128],
                            w_sb[:, kc, 4 * 128 : 6 * 128],
                            start=(kc == 0),
                            stop=(kc == KC - 1),
                        )
                    nc.vector.tensor_add(
                        v_tiles[b][:, half * 4 + t, :], psv, bv_sb
                    )
                    if fi < len(fillers):
                        fillers[fi]()
                        fi += 1
                while fi < len(fillers):
                    fillers[fi]()
                    fi += 1

            def score_chunk(b, h, ki, c0, n, with_mask):
                """One scores matmul + exp for k-chunk ki, q range
                [c0, c0+n); optionally applies the diagonal causal mask."""
                qT = qk_tiles[b][h]
                kT = qk_tiles[b][2 + h]
                e_sb = e_tiles[(b, h)]
                lo = ki * 128
                pss = ps_misc.tile(
                    [128, n],
                    F32,
                    tag="m",
                    padded_shape=[128, NSL],
                    name=f"pss_{b}_{h}_{ki}_{c0}",
                )
                nc.tensor.matmul(
                    pss, kT[:, lo : lo + 128], qT[:, c0 : c0 + n],
                    start=True, stop=True,
                )
                nc.scalar.activation(
                    e_sb[:, ki, c0 : c0 + n], pss, Exp, scale=SCALE
                )
                if with_mask:
                    nc.vector.tensor_mul(
                        e_sb[:, ki, lo : lo + 128],
                        e_sb[:, ki, lo : lo + 128],
                        mask_sb,
                    )

            def scores_a_fillers(b, h):
                # half-0 triangle: needs only proj(b, 0)
                e_tiles[(b, h)] = e_pool.tile(
                    [128, 8, S], BF16, tag="e", name=f"e_{b}_{h}"
                )
                return [
                    (lambda ki=ki: score_chunk(b, h, ki, ki * 128,
                                               NSL - ki * 128, True))
                    for ki in range(4)
                ]

            def scores_b_fillers(b, h):
                # the rest: k chunks 0..3 over q half 1, and k chunks 4..7
                fs = [
                    (lambda ki=ki: score_chunk(b, h, ki, NSL, NSL, False))
                    for ki in range(4)
                ]
                fs += [
                    (lambda ki=ki: score_chunk(b, h, ki, ki * 128,
                                               S - ki * 128, True))
                    for ki in range(4, 8)
                ]
                return fs

            def avones(b, h):
                """AV + softmax-denominator accumulation and epilogue for
                one head: per 512-wide q half, accumulate over k chunks with
                causally-clipped q ranges."""
                e_sb = e_tiles.pop((b, h))
                v_sb = v_tiles[b]
                po = [
                    ps_o_pool.tile(
                        [128, NSL], F32, tag="o", name=f"po_{b}_{h}_{x}"
                    )
                    for x in range(2)
                ]
                for half in range(2):
                    q0 = half * NSL
                    kis = [ki for ki in range(8) if ki * 128 < q0 + NSL]
                    for idx, ki in enumerate(kis):
                        a = max(ki * 128, q0)
                        nc.tensor.matmul(
                            po[half][:, a - q0 : NSL],
                            v_sb[:, ki, h * HD : (h + 1) * HD],
                            e_sb[:, ki, a : q0 + NSL],
                            start=(idx == 0),
                            stop=(idx == len(kis) - 1),
                            skip_group_check=True,
                        )
                    pd = ps_d_pool.tile(
                        [128, NSL], F32, tag="d", name=f"pd_{b}_{h}_{half}"
                    )
                    for idx, ki in enumerate(kis):
                        a = max(ki * 128, q0)
                        nc.tensor.matmul(
                            pd[:, a - q0 : NSL],
                            ones_sb,
                            e_sb[:, ki, a : q0 + NSL],
                            start=(idx == 0),
                            stop=(idx == len(kis) - 1),
                            skip_group_check=True,
                        )
                    rc = rcp_pool.tile(
                        [128, NSL], F32, tag="rc", name=f"rc_{b}_{h}_{half}"
                    )
                    nc.vector.reciprocal_approx_fast(out=rc, in_=pd)
                    o = out_pool.tile(
                        [128, NSL], BF16, tag="o", name=f"o_{b}_{h}_{half}"
                    )
                    nc.vector.tensor_mul(o, po[half], rc)
                    nc.scalar.dma_start(
                        out=out_ap[h, :, b, q0 : q0 + NSL], in_=o
                    )

            # ---- schedule: proj segments with scores pipelined in ----
            xs = [load_x(0), load_x(1)]
            proj(0, xs[0], [])
            for b in range(B):
                seg1 = 2 * b + 1
                if seg1 + 1 < NSEG:
                    xs.append(load_x(seg1 + 1))
                proj(seg1, xs[seg1],
                     scores_a_fillers(b, 0) + scores_a_fillers(b, 1))
                seg2 = seg1 + 1
                if seg2 < NSEG:
                    if seg2 + 1 < NSEG:
                        xs.append(load_x(seg2 + 1))
                    proj(seg2, xs[seg2],
                         scores_b_fillers(b, 0) + scores_b_fillers(b, 1))
                else:
                    for f in scores_b_fillers(b, 0) + scores_b_fillers(b, 1):
                        f()
                avones(b, 0)
                avones(b, 1)

    nc.compile()
    return nc


def _prep_shared(hidden_states):
    x2 = np.ascontiguousarray(hidden_states.reshape(T, D).T)  # [D, T]
    x_host = np.ascontiguousarray(
        x2.astype(BF).reshape(KC, 128, NSEG, NSL).transpose(1, 2, 0, 3)
    )  # [128, NSEG, KC, NSL]

    inv = 1.0 / (ROPE_BASE ** (np.arange(0, HD, 2, dtype=np.float64) / HD))
    f = np.outer(inv, np.arange(S, dtype=np.float64))  # [64, S]
    cosT = np.concatenate([np.cos(f), np.cos(f)], axis=0).astype(np.float32)
    sinS = np.concatenate([np.sin(f), np.sin(f)], axis=0).astype(np.float32)

    p = np.arange(128)[:, None]
    j = np.arange(128)[None, :]
    mask0 = np.ascontiguousarray((j >= p).astype(np.float32))

    # rotate_half as a matmul: out = lhsT.T @ rhs with lhsT = rotT gives
    # (R @ q)[i] = -q[i+64] (i<64), q[i-64] (i>=64)
    rotT = np.zeros((128, 128), np.float32)
    rotT[np.arange(64), np.arange(64) + 64] = 1.0
    rotT[np.arange(64) + 64, np.arange(64)] = -1.0
    return x_host, cosT, sinS, mask0, rotT.astype(BF)


def _core_rows(c):
    h0, h1 = 2 * c, 2 * c + 1
    rows = []
    for part in range(3):  # q, k, v blocks
        for h in (h0, h1):
            base = h * 3 * HD + part * HD
            rows.extend(range(base, base + HD))
    return np.asarray(rows)


def _prep_core(w_qkv, b_qkv, c):
    rows = _core_rows(c)
    wT = np.ascontiguousarray(w_qkv[rows, :].T)  # [D, 768]
    w_host = np.ascontiguousarray(
        wT.astype(BF).reshape(KC, 128, 6 * HD).transpose(1, 0, 2)
    )  # [128, KC, 768]
    b_sel = b_qkv[rows]
    bqk = np.ascontiguousarray(b_sel[: 4 * 128].reshape(4, 128).T)  # [128, 4]
    bv = np.ascontiguousarray(
        np.broadcast_to(b_sel[4 * 128 :], (128, 2 * HD))
    )  # [128, 256]
    return w_host, bqk, bv


def _make_in_maps(hidden_states, w_qkv, b_qkv):
    x_host, cosT, sinS, mask0, rotT = _prep_shared(hidden_states)
    ones = np.ones((128, 128), BF)
    in_maps = []
    for c in range(NCORES):
        w_host, bqk, bv = _prep_core(w_qkv, b_qkv, c)
        in_maps.append(
            {
                "x": x_host,
                "w": w_host,
                "bqk": bqk,
                "bv": bv,
                "cosT": cosT,
                "sinS": sinS,
                "mask0": mask0,
                "rotT": rotT,
                "ones": ones,
            }
        )
    return in_maps


def _assemble(results):
    outs = np.stack(
        [np.asarray(results[c]["out"]).astype(np.float32) for c in range(NCORES)]
    )
    # [NCORES, HPC, HD, B, S] -> [B, S, H*HD]
    return np.ascontiguousarray(
        outs.reshape(H, HD, B, S).transpose(2, 3, 0, 1).reshape(B, S, D)
    )


def run(hidden_states, w_qkv, b_qkv, trace=False):
    from concourse.bass_utils import run_bass_kernel_spmd

    if "nc" not in _CACHE:
        _CACHE["nc"] = _build_program()
    nc = _CACHE["nc"]
    in_maps = _make_in_maps(
        np.asarray(hidden_states, dtype=np.float32),
        np.asarray(w_qkv, dtype=np.float32),
        np.asarray(b_qkv, dtype=np.float32),
    )
    res = run_bass_kernel_spmd(
        nc, in_maps, core_ids=list(range(NCORES)), trace=trace
    )
    out = _assemble(res.results)
    return out, res


def kernel(hidden_states, w_qkv, b_qkv):
    trace = os.environ.get("KERNEL_TRACE", "0") == "1"
    out, _res = run(hidden_states, w_qkv, b_qkv, trace=trace)
    return out


# revision 9
# speedup vs baseline: 1.3836x; 1.0243x over previous
"""GPT-NeoX attention (B=4, S=1024, D=2048, H=16) on 8 TRN2 NeuronCores.

Tensor-parallel over heads: 2 heads per core. Each core computes its slice
of the fused QKV projection, RoPE, causal attention, and writes the
transposed per-head output [hd, S] in bf16; the host concatenates heads.

All on-chip matmuls use bf16 operands (1 cycle/row at any free size) with
fp32 PSUM accumulation. Attention computes only the valid causal region:
  - scores sT[k_chunk, q] for q >= k_chunk_start only
  - AV / ones-denominator accumulate per 512-wide q half with per-k-chunk
    clipped q ranges (element-wise PSUM accumulation handles the ragged
    start offsets)
Scores+exp for batch b are software-pipelined into the projection matmul
chains of the next segment so the scalar-engine exp latency never stalls
the in-order PE queue.
"""

import os

import numpy as np
import ml_dtypes

import concourse.bass as bass
import concourse.tile as tile
from concourse import bacc, mybir

# Problem constants (contract: nn_GPTNeoXAttention, fixed shapes)
B, S, D = 4, 1024, 2048
H = 16
HD = 128  # head dim
NCORES = 8
HPC = H // NCORES  # heads per core
ROPE_BASE = 10000.0
T = B * S  # 4096 tokens
KC = D // 128  # 16 contraction chunks of the model dim
NSL = 512  # token-slice width per projection segment
NSEG = B * 2
SCALE = 1.0 / float(np.sqrt(HD))

F32 = mybir.dt.float32
BF16 = mybir.dt.bfloat16
BF = ml_dtypes.bfloat16

_CACHE = {}


def _build_program():
    nc = bacc.Bacc(
        "TRN2", target_bir_lowering=False, debug=False, num_devices=NCORES
    )

    x_d = nc.dram_tensor("x", [128, NSEG, KC, NSL], BF16, kind="ExternalInput")
    w_d = nc.dram_tensor("w", [128, KC, 6 * HD], BF16, kind="ExternalInput")
    bqk_d = nc.dram_tensor("bqk", [128, 4], F32, kind="ExternalInput")
    bv_d = nc.dram_tensor("bv", [128, 2 * HD], F32, kind="ExternalInput")
    cos_d = nc.dram_tensor("cosT", [128, S], BF16, kind="ExternalInput")
    sin_d = nc.dram_tensor("sinS", [128, S], BF16, kind="ExternalInput")
    mask_d = nc.dram_tensor("mask0", [128, 128], F32, kind="ExternalInput")
    ones_d = nc.dram_tensor("ones", [128, 128], BF16, kind="ExternalInput")
    out_d = nc.dram_tensor("out", [HPC, HD, B, S], BF16, kind="ExternalOutput")

    x_ap = x_d.ap()
    w_ap = w_d.ap()
    out_ap = out_d.ap()

    Exp = mybir.ActivationFunctionType.Exp
    Identity = mybir.ActivationFunctionType.Identity

    with tile.TileContext(nc) as tc:
        with (
            tc.tile_pool(name="singles", bufs=1) as singles,
            tc.tile_pool(name="xin", bufs=2) as xin_pool,
            tc.tile_pool(name="qk", bufs=8) as qk_pool,
            tc.tile_pool(name="vp", bufs=2) as v_pool,
            tc.tile_pool(name="ep", bufs=2) as e_pool,
            tc.tile_pool(name="qbp", bufs=3) as qb_pool,
            tc.tile_pool(name="tmp", bufs=3) as tmp_pool,
            tc.tile_pool(name="outp", bufs=3) as out_pool,
            tc.tile_pool(name="rcp", bufs=2) as rcp_pool,
            tc.tile_pool(name="ps_proj", bufs=2, space="PSUM") as ps_proj,
            tc.tile_pool(name="ps_misc", bufs=3, space="PSUM") as ps_misc,
            tc.tile_pool(name="ps_o", bufs=2, space="PSUM") as ps_o_pool,
            tc.tile_pool(name="ps_d", bufs=1, space="PSUM") as ps_d_pool,
        ):
            w_sb = singles.tile([128, KC, 6 * HD], BF16)
            for kc in range(KC):
                nc.scalar.dma_start(out=w_sb[:, kc, :], in_=w_ap[:, kc, :])
            cos_sb = singles.tile([128, S], BF16)
            nc.gpsimd.dma_start(out=cos_sb, in_=cos_d.ap())
            sin_sb = singles.tile([128, S], BF16)
            nc.gpsimd.dma_start(out=sin_sb, in_=sin_d.ap())
            bqk_sb = singles.tile([128, 4], F32)
            nc.gpsimd.dma_start(out=bqk_sb, in_=bqk_d.ap())
            bv_sb = singles.tile([128, 2 * HD], F32)
            nc.gpsimd.dma_start(out=bv_sb, in_=bv_d.ap())
            mask_sb = singles.tile([128, 128], F32)
            nc.gpsimd.dma_start(out=mask_sb, in_=mask_d.ap())
            ones_sb = singles.tile([128, 128], BF16)
            nc.gpsimd.dma_start(out=ones_sb, in_=ones_d.ap())

            qk_tiles = {}  # b -> [qh0, qh1, kh0, kh1] feature-major tiles
            v_tiles = {}  # b -> natural-layout v tile
            e_tiles = {}  # (b, h) -> exp'd scores, k-chunk-major

            def load_x(seg):
                xsb = xin_pool.tile(
                    [128, KC, NSL], BF16, tag="x", name=f"xsb_{seg}"
                )
                # finer chunks on the first segment so the first matmul
                # chain can start as soon as the leading kc slices land
                csz = 2 if seg == 0 else 4
                for c in range(KC // csz):
                    nc.sync.dma_start(
                        out=xsb[:, csz * c : csz * (c + 1), :],
                        in_=x_ap[:, seg, csz * c : csz * (c + 1), :],
                    )
                return xsb

            def proj(seg, xsb, fillers):
                """QKV projection for one 512-token segment; fillers are
                emitted between matmul chains to keep other engines fed."""
                b, half = divmod(seg, 2)
                if half == 0:
                    qk_tiles[b] = [
                        qk_pool.tile(
                            [128, S], BF16, tag="qkt", name=f"qkt_{b}_{i}"
                        )
                        for i in range(4)
                    ]
                    v_tiles[b] = v_pool.tile(
                        [128, S // 128, 2 * HD], BF16, tag="v", name=f"v_{b}"
                    )
                fi = 0
                sl = slice(half * NSL, (half + 1) * NSL)
                for m in range(4):
                    ps = ps_proj.tile(
                        [128, NSL], F32, tag="ps", name=f"psqk_{seg}_{m}"
                    )
                    for kc in range(KC):
                        nc.tensor.matmul(
                            ps,
                            w_sb[:, kc, m * 128 : (m + 1) * 128],
                            xsb[:, kc, :],
                            start=(kc == 0),
                            stop=(kc == KC - 1),
                        )
                    qb = qb_pool.tile(
                        [128, NSL], BF16, tag="qb", name=f"qb_{seg}_{m}"
                    )
                    nc.scalar.activation(
                        qb, ps, Identity, bias=bqk_sb[:, m : m + 1], scale=1.0
                    )
                    # rotate_half via partition-shift SBUF->SBUF DMAs on the
                    # otherwise-idle gpsimd queue; the sign lives in sin_sb
                    rsh = tmp_pool.tile(
                        [128, NSL], BF16, tag="r", name=f"rsh_{seg}_{m}"
                    )
                    nc.gpsimd.dma_start(out=rsh[0:64, :], in_=qb[64:128, :])
                    nc.gpsimd.dma_start(out=rsh[64:128, :], in_=qb[0:64, :])
                    dst = qk_tiles[b][m][:, sl]
                    tmp2 = tmp_pool.tile(
                        [128, NSL], BF16, tag="t", name=f"tmp_{seg}_{m}"
                    )
                    nc.vector.tensor_mul(tmp2, rsh, sin_sb[:, sl])
                    nc.vector.tensor_mul(dst, qb, cos_sb[:, sl])
                    nc.vector.tensor_add(dst, dst, tmp2)
                    if fi < len(fillers):
                        fillers[fi]()
                        fi += 1
                for t in range(NSL // 128):
                    psv = ps_proj.tile(
                        [128, 2 * HD],
                        F32,
                        tag="ps",
                        padded_shape=[128, NSL],
                        name=f"psv_{seg}_{t}",
                    )
                    for kc in range(KC):
                        nc.tensor.matmul(
                            psv,
                            xsb[:, kc, t * 128 : (t + 1) * 128],
                            w_sb[:, kc, 4 * 128 : 6 * 128],
                            start=(kc == 0),
                            stop=(kc == KC - 1),
                        )
                    nc.vector.tensor_add(
                        v_tiles[b][:, half * 4 + t, :], psv, bv_sb
                    )
                    if fi < len(fillers):
                        fillers[fi]()
                        fi += 1
                while fi < len(fillers):
                    fillers[fi]()
                    fi += 1

            def score_chunk(b, h, ki, c0, n, with_mask):
                """One scores matmul + exp for k-chunk ki, q range
                [c0, c0+n); optionally applies the diagonal causal mask."""
                qT = qk_tiles[b][h]
                kT = qk_tiles[b][2 + h]
                e_sb = e_tiles[(b, h)]
                lo = ki * 128
                pss = ps_misc.tile(
                    [128, n],
                    F32,
                    tag="m",
                    padded_shape=[128, NSL],
                    name=f"pss_{b}_{h}_{ki}_{c0}",
                )
                nc.tensor.matmul(
                    pss, kT[:, lo : lo + 128], qT[:, c0 : c0 + n],
                    start=True, stop=True,
                )
                nc.scalar.activation(
                    e_sb[:, ki, c0 : c0 + n], pss, Exp, scale=SCALE
                )
                if with_mask:
                    nc.vector.tensor_mul(
                        e_sb[:, ki, lo : lo + 128],
                        e_sb[:, ki, lo : lo + 128],
                        mask_sb,
                    )

            def scores_a_fillers(b, h):
                # half-0 triangle: needs only proj(b, 0)
                e_tiles[(b, h)] = e_pool.tile(
                    [128, 8, S], BF16, tag="e", name=f"e_{b}_{h}"
                )
                return [
                    (lambda ki=ki: score_chunk(b, h, ki, ki * 128,
                                               NSL - ki * 128, True))
                    for ki in range(4)
                ]

            def scores_b_fillers(b, h):
                # the rest: k chunks 0..3 over q half 1, and k chunks 4..7
                fs = [
                    (lambda ki=ki: score_chunk(b, h, ki, NSL, NSL, False))
                    for ki in range(4)
                ]
                fs += [
                    (lambda ki=ki: score_chunk(b, h, ki, ki * 128,
                                               S - ki * 128, True))
                    for ki in range(4, 8)
                ]
                return fs

            def avones(b, h):
                """AV + softmax-denominator accumulation and epilogue for
                one head: per 512-wide q half, accumulate over k chunks with
                causally-clipped q ranges."""
                e_sb = e_tiles.pop((b, h))
                v_sb = v_tiles[b]
                po = [
                    ps_o_pool.tile(
                        [128, NSL], F32, tag="o", name=f"po_{b}_{h}_{x}"
                    )
                    for x in range(2)
                ]
                for half in range(2):
                    q0 = half * NSL
                    kis = [ki for ki in range(8) if ki * 128 < q0 + NSL]
                    for idx, ki in enumerate(kis):
                        a = max(ki * 128, q0)
                        nc.tensor.matmul(
                            po[half][:, a - q0 : NSL],
                            v_sb[:, ki, h * HD : (h + 1) * HD],
                            e_sb[:, ki, a : q0 + NSL],
                            start=(idx == 0),
                            stop=(idx == len(kis) - 1),
                            skip_group_check=True,
                        )
                    pd = ps_d_pool.tile(
                        [128, NSL], F32, tag="d", name=f"pd_{b}_{h}_{half}"
                    )
                    for idx, ki in enumerate(kis):
                        a = max(ki * 128, q0)
                        nc.tensor.matmul(
                            pd[:, a - q0 : NSL],
                            ones_sb,
                            e_sb[:, ki, a : q0 + NSL],
                            start=(idx == 0),
                            stop=(idx == len(kis) - 1),
                            skip_group_check=True,
                        )
                    rc = rcp_pool.tile(
                        [128, NSL], F32, tag="rc", name=f"rc_{b}_{h}_{half}"
                    )
                    nc.vector.reciprocal_approx_fast(out=rc, in_=pd)
                    o = out_pool.tile(
                        [128, NSL], BF16, tag="o", name=f"o_{b}_{h}_{half}"
                    )
                    nc.vector.tensor_mul(o, po[half], rc)
                    nc.scalar.dma_start(
                        out=out_ap[h, :, b, q0 : q0 + NSL], in_=o
                    )

            # ---- schedule: proj segments with scores pipelined in ----
            xs = [load_x(0), load_x(1)]
            proj(0, xs[0], [])
            for b in range(B):
                seg1 = 2 * b + 1
                if seg1 + 1 < NSEG:
                    xs.append(load_x(seg1 + 1))
                proj(seg1, xs[seg1],
                     scores_a_fillers(b, 0) + scores_a_fillers(b, 1))
                seg2 = seg1 + 1
                if seg2 < NSEG:
                    if seg2 + 1 < NSEG:
                        xs.append(load_x(seg2 + 1))
                    proj(seg2, xs[seg2],
                         scores_b_fillers(b, 0) + scores_b_fillers(b, 1))
                else:
                    for f in scores_b_fillers(b, 0) + scores_b_fillers(b, 1):
                        f()
                avones(b, 0)
                avones(b, 1)

    nc.compile()
    return nc


def _prep_shared(hidden_states):
    x2 = np.ascontiguousarray(hidden_states.reshape(T, D).T)  # [D, T]
    x_host = np.ascontiguousarray(
        x2.astype(BF).reshape(KC, 128, NSEG, NSL).transpose(1, 2, 0, 3)
    )  # [128, NSEG, KC, NSL]

    inv = 1.0 / (ROPE_BASE ** (np.arange(0, HD, 2, dtype=np.float64) / HD))
    f = np.outer(inv, np.arange(S, dtype=np.float64))  # [64, S]
    cosT = np.concatenate([np.cos(f), np.cos(f)], axis=0).astype(BF)
    # sign of rotate_half folded in: rot(q)[p] = -q[p+64] (p<64), q[p-64]
    sin_half = np.sin(f)
    sinS = np.concatenate([-sin_half, sin_half], axis=0).astype(BF)

    p = np.arange(128)[:, None]
    j = np.arange(128)[None, :]
    mask0 = np.ascontiguousarray((j >= p).astype(np.float32))
    return x_host, cosT, sinS, mask0


def _core_rows(c):
    h0, h1 = 2 * c, 2 * c + 1
    rows = []
    for part in range(3):  # q, k, v blocks
        for h in (h0, h1):
            base = h * 3 * HD + part * HD
            rows.extend(range(base, base + HD))
    return np.asarray(rows)


def _prep_core(w_qkv, b_qkv, c):
    rows = _core_rows(c)
    wT = np.ascontiguousarray(w_qkv[rows, :].T)  # [D, 768]
    w_host = np.ascontiguousarray(
        wT.astype(BF).reshape(KC, 128, 6 * HD).transpose(1, 0, 2)
    )  # [128, KC, 768]
    b_sel = b_qkv[rows]
    bqk = np.ascontiguousarray(b_sel[: 4 * 128].reshape(4, 128).T)  # [128, 4]
    bv = np.ascontiguousarray(
        np.broadcast_to(b_sel[4 * 128 :], (128, 2 * HD))
    )  # [128, 256]
    return w_host, bqk, bv


def _make_in_maps(hidden_states, w_qkv, b_qkv):
    x_host, cosT, sinS, mask0 = _prep_shared(hidden_states)
    ones = np.ones((128, 128), BF)
    in_maps = []
    for c in range(NCORES):
        w_host, bqk, bv = _prep_core(w_qkv, b_qkv, c)
        in_maps.append(
            {
                "x": x_host,
                "w": w_host,
                "bqk": bqk,
                "bv": bv,
                "cosT": cosT,
                "sinS": sinS,
                "mask0": mask0,
                "ones": ones,
            }
        )
    return in_maps


def _assemble(results):
    outs = np.stack(
        [np.asarray(results[c]["out"]).astype(np.float32) for c in range(NCORES)]
    )
    # [NCORES, HPC, HD, B, S] -> [B, S, H*HD]
    return np.ascontiguousarray(
        outs.reshape(H, HD, B, S).transpose(2, 3, 0, 1).reshape(B, S, D)
    )


def run(hidden_states, w_qkv, b_qkv, trace=False):
    from concourse.bass_utils import run_bass_kernel_spmd

    if "nc" not in _CACHE:
        _CACHE["nc"] = _build_program()
    nc = _CACHE["nc"]
    in_maps = _make_in_maps(
        np.asarray(hidden_states, dtype=np.float32),
        np.asarray(w_qkv, dtype=np.float32),
        np.asarray(b_qkv, dtype=np.float32),
    )
    res = run_bass_kernel_spmd(
        nc, in_maps, core_ids=list(range(NCORES)), trace=trace
    )
    out = _assemble(res.results)
    return out, res


def kernel(hidden_states, w_qkv, b_qkv):
    trace = os.environ.get("KERNEL_TRACE", "0") == "1"
    out, _res = run(hidden_states, w_qkv, b_qkv, trace=trace)
    return out


# revision 16
# speedup vs baseline: 1.4295x; 1.0331x over previous
"""GPT-NeoX attention (B=4, S=1024, D=2048, H=16) on 8 TRN2 NeuronCores.

Tensor-parallel over heads: 2 heads per core. Each core computes its slice
of the fused QKV projection, RoPE, causal attention, and writes the
transposed per-head output [hd, S] in bf16; the host concatenates heads.

All on-chip matmuls use bf16 operands (1 cycle/row at any free size) with
fp32 PSUM accumulation. Attention computes only the valid causal region:
  - scores sT[k_chunk, q] for q >= k_chunk_start only
  - AV / ones-denominator accumulate per 512-wide q half with per-k-chunk
    clipped q ranges (element-wise PSUM accumulation handles the ragged
    start offsets)
Scores+exp for batch b are software-pipelined into the projection matmul
chains of the next segment so the scalar-engine exp latency never stalls
the in-order PE queue.
"""

import os

import numpy as np
import ml_dtypes

import concourse.bass as bass
import concourse.tile as tile
from concourse import bacc, mybir

# Problem constants (contract: nn_GPTNeoXAttention, fixed shapes)
B, S, D = 4, 1024, 2048
H = 16
HD = 128  # head dim
NCORES = 8
HPC = H // NCORES  # heads per core
ROPE_BASE = 10000.0
T = B * S  # 4096 tokens
KC = D // 128  # 16 contraction chunks of the model dim
NSL = 512  # token-slice width per projection segment
NSEG = B * 2
SCALE = 1.0 / float(np.sqrt(HD))

F32 = mybir.dt.float32
BF16 = mybir.dt.bfloat16
BF = ml_dtypes.bfloat16

_CACHE = {}


def _build_program():
    nc = bacc.Bacc(
        "TRN2", target_bir_lowering=False, debug=False, num_devices=NCORES
    )

    x_d = nc.dram_tensor("x", [128, NSEG, KC, NSL], BF16, kind="ExternalInput")
    wqk_d = nc.dram_tensor("wqk", [128, 4, KC, 128], BF16, kind="ExternalInput")
    wv_d = nc.dram_tensor("wv", [128, KC, 2 * HD], BF16, kind="ExternalInput")
    bqk_d = nc.dram_tensor("bqk", [128, 4], F32, kind="ExternalInput")
    bv_d = nc.dram_tensor("bv", [128, 2 * HD], F32, kind="ExternalInput")
    cos_d = nc.dram_tensor("cosT", [128, S], BF16, kind="ExternalInput")
    sin_d = nc.dram_tensor("sinS", [128, S], BF16, kind="ExternalInput")
    mask_d = nc.dram_tensor("mask0", [128, 128], F32, kind="ExternalInput")
    ones_d = nc.dram_tensor("ones", [128, 128], BF16, kind="ExternalInput")
    out_d = nc.dram_tensor("out", [HPC, HD, B, S], BF16, kind="ExternalOutput")

    x_ap = x_d.ap()
    wqk_ap = wqk_d.ap()
    wv_ap = wv_d.ap()
    out_ap = out_d.ap()

    Exp = mybir.ActivationFunctionType.Exp
    Identity = mybir.ActivationFunctionType.Identity

    with tile.TileContext(nc) as tc:
        with (
            tc.tile_pool(name="singles", bufs=1) as singles,
            tc.tile_pool(name="xin", bufs=2) as xin_pool,
            tc.tile_pool(name="qk", bufs=8) as qk_pool,
            tc.tile_pool(name="vp", bufs=2) as v_pool,
            tc.tile_pool(name="ep", bufs=2) as e_pool,
            tc.tile_pool(name="qbp", bufs=3) as qb_pool,
            tc.tile_pool(name="tmp", bufs=3) as tmp_pool,
            tc.tile_pool(name="outp", bufs=3) as out_pool,
            tc.tile_pool(name="rcp", bufs=2) as rcp_pool,
            tc.tile_pool(name="ps_proj", bufs=2, space="PSUM") as ps_proj,
            tc.tile_pool(name="ps_misc", bufs=3, space="PSUM") as ps_misc,
            tc.tile_pool(name="ps_o", bufs=2, space="PSUM") as ps_o_pool,
            tc.tile_pool(name="ps_d", bufs=1, space="PSUM") as ps_d_pool,
        ):
            # m-block-major weight loads: the first projection chain only
            # needs its own 0.5 MB block, so compute starts ~immediately
            wqk_sb = singles.tile([128, 4, KC, 128], BF16)
            for c in range(4):  # finer chunks for the very first chain
                nc.scalar.dma_start(
                    out=wqk_sb[:, 0, 4 * c : 4 * c + 4, :],
                    in_=wqk_ap[:, 0, 4 * c : 4 * c + 4, :],
                )
            for m in range(1, 4):
                nc.scalar.dma_start(
                    out=wqk_sb[:, m, :, :], in_=wqk_ap[:, m, :, :]
                )
            wv_sb = singles.tile([128, KC, 2 * HD], BF16)
            nc.scalar.dma_start(out=wv_sb, in_=wv_d.ap())
            cos_sb = singles.tile([128, S], BF16)
            nc.gpsimd.dma_start(out=cos_sb, in_=cos_d.ap())
            sin_sb = singles.tile([128, S], BF16)
            nc.gpsimd.dma_start(out=sin_sb, in_=sin_d.ap())
            bqk_sb = singles.tile([128, 4], F32)
            nc.gpsimd.dma_start(out=bqk_sb, in_=bqk_d.ap())
            bv_sb = singles.tile([128, 2 * HD], F32)
            nc.gpsimd.dma_start(out=bv_sb, in_=bv_d.ap())
            mask_sb = singles.tile([128, 128], F32)
            nc.gpsimd.dma_start(out=mask_sb, in_=mask_d.ap())
            ones_sb = singles.tile([128, 128], BF16)
            nc.gpsimd.dma_start(out=ones_sb, in_=ones_d.ap())

            qk_tiles = {}  # b -> [qh0, qh1, kh0, kh1] feature-major tiles
            v_tiles = {}  # b -> natural-layout v tile
            e_tiles = {}  # (b, h) -> exp'd scores, k-chunk-major

            def load_x(seg):
                xsb = xin_pool.tile(
                    [128, KC, NSL], BF16, tag="x", name=f"xsb_{seg}"
                )
                # finer chunks on the first segment so the first matmul
                # chain can start as soon as the leading kc slices land
                csz = 2 if seg == 0 else 4
                for c in range(KC // csz):
                    nc.sync.dma_start(
                        out=xsb[:, csz * c : csz * (c + 1), :],
                        in_=x_ap[:, seg, csz * c : csz * (c + 1), :],
                    )
                return xsb

            def proj(seg, xsb, fillers):
                """QKV projection for one 512-token segment; fillers are
                emitted between matmul chains to keep other engines fed."""
                b, half = divmod(seg, 2)
                if half == 0:
                    qk_tiles[b] = [
                        qk_pool.tile(
                            [128, S], BF16, tag="qkt", name=f"qkt_{b}_{i}"
                        )
                        for i in range(4)
                    ]
                    v_tiles[b] = v_pool.tile(
                        [128, S // 128, 2 * HD], BF16, tag="v", name=f"v_{b}"
                    )
                fi = 0
                sl = slice(half * NSL, (half + 1) * NSL)
                for m in range(4):
                    ps = ps_proj.tile(
                        [128, NSL], F32, tag="ps", name=f"psqk_{seg}_{m}"
                    )
                    for kc in range(KC):
                        nc.tensor.matmul(
                            ps,
                            wqk_sb[:, m, kc, :],
                            xsb[:, kc, :],
                            start=(kc == 0),
                            stop=(kc == KC - 1),
                        )
                    qb = qb_pool.tile(
                        [128, NSL], BF16, tag="qb", name=f"qb_{seg}_{m}"
                    )
                    nc.scalar.activation(
                        qb, ps, Identity, bias=bqk_sb[:, m : m + 1], scale=1.0
                    )
                    # rotate_half via partition-shift SBUF->SBUF DMAs on the
                    # otherwise-idle gpsimd queue; the sign lives in sin_sb
                    rsh = tmp_pool.tile(
                        [128, NSL], BF16, tag="r", name=f"rsh_{seg}_{m}"
                    )
                    nc.gpsimd.dma_start(out=rsh[0:64, :], in_=qb[64:128, :])
                    nc.gpsimd.dma_start(out=rsh[64:128, :], in_=qb[0:64, :])
                    dst = qk_tiles[b][m][:, sl]
                    tmp2 = tmp_pool.tile(
                        [128, NSL], BF16, tag="t", name=f"tmp_{seg}_{m}"
                    )
                    nc.vector.tensor_mul(tmp2, rsh, sin_sb[:, sl])
                    nc.vector.tensor_mul(dst, qb, cos_sb[:, sl])
                    nc.vector.tensor_add(dst, dst, tmp2)
                    if fi < len(fillers):
                        fillers[fi]()
                        fi += 1
                for t in range(NSL // 128):
                    psv = ps_proj.tile(
                        [128, 2 * HD],
                        F32,
                        tag="ps",
                        padded_shape=[128, NSL],
                        name=f"psv_{seg}_{t}",
                    )
                    for kc in range(KC):
                        nc.tensor.matmul(
                            psv,
                            xsb[:, kc, t * 128 : (t + 1) * 128],
                            wv_sb[:, kc, :],
                            start=(kc == 0),
                            stop=(kc == KC - 1),
                        )
                    nc.vector.tensor_add(
                        v_tiles[b][:, half * 4 + t, :], psv, bv_sb
                    )
                    if fi < len(fillers):
                        fillers[fi]()
                        fi += 1
                while fi < len(fillers):
                    fillers[fi]()
                    fi += 1

            def score_chunk(b, h, ki, c0, n, with_mask):
                """One scores matmul + exp for k-chunk ki, q range
                [c0, c0+n); optionally applies the diagonal causal mask."""
                qT = qk_tiles[b][h]
                kT = qk_tiles[b][2 + h]
                e_sb = e_tiles[(b, h)]
                lo = ki * 128
                pss = ps_misc.tile(
                    [128, n],
                    F32,
                    tag="m",
                    padded_shape=[128, NSL],
                    name=f"pss_{b}_{h}_{ki}_{c0}",
                )
                nc.tensor.matmul(
                    pss, kT[:, lo : lo + 128], qT[:, c0 : c0 + n],
                    start=True, stop=True,
                )
                nc.scalar.activation(
                    e_sb[:, ki, c0 : c0 + n], pss, Exp, scale=SCALE
                )
                if with_mask:
                    nc.vector.tensor_mul(
                        e_sb[:, ki, lo : lo + 128],
                        e_sb[:, ki, lo : lo + 128],
                        mask_sb,
                    )

            def scores_a_fillers(b, h):
                # half-0 triangle: needs only proj(b, 0)
                e_tiles[(b, h)] = e_pool.tile(
                    [128, 8, S], BF16, tag="e", name=f"e_{b}_{h}"
                )
                return [
                    (lambda ki=ki: score_chunk(b, h, ki, ki * 128,
                                               NSL - ki * 128, True))
                    for ki in range(4)
                ]

            def scores_b_fillers(b, h):
                # the rest: k chunks 0..3 over q half 1, and k chunks 4..7
                fs = [
                    (lambda ki=ki: score_chunk(b, h, ki, NSL, NSL, False))
                    for ki in range(4)
                ]
                fs += [
                    (lambda ki=ki: score_chunk(b, h, ki, ki * 128,
                                               S - ki * 128, True))
                    for ki in range(4, 8)
                ]
                return fs

            def avones(b, h):
                """AV + softmax-denominator accumulation and epilogue for
                one head: per 512-wide q half, accumulate over k chunks with
                causally-clipped q ranges."""
                e_sb = e_tiles.pop((b, h))
                v_sb = v_tiles[b]
                po = [
                    ps_o_pool.tile(
                        [128, NSL], F32, tag="o", name=f"po_{b}_{h}_{x}"
                    )
                    for x in range(2)
                ]
                for half in range(2):
                    q0 = half * NSL
                    kis = [ki for ki in range(8) if ki * 128 < q0 + NSL]
                    for idx, ki in enumerate(kis):
                        a = max(ki * 128, q0)
                        nc.tensor.matmul(
                            po[half][:, a - q0 : NSL],
                            v_sb[:, ki, h * HD : (h + 1) * HD],
                            e_sb[:, ki, a : q0 + NSL],
                            start=(idx == 0),
                            stop=(idx == len(kis) - 1),
                            skip_group_check=True,
                        )
                    pd = ps_d_pool.tile(
                        [128, NSL], F32, tag="d", name=f"pd_{b}_{h}_{half}"
                    )
                    for idx, ki in enumerate(kis):
                        a = max(ki * 128, q0)
                        nc.tensor.matmul(
                            pd[:, a - q0 : NSL],
                            ones_sb,
                            e_sb[:, ki, a : q0 + NSL],
                            start=(idx == 0),
                            stop=(idx == len(kis) - 1),
                            skip_group_check=True,
                        )
                    rc = rcp_pool.tile(
                        [128, NSL], F32, tag="rc", name=f"rc_{b}_{h}_{half}"
                    )
                    nc.vector.reciprocal_approx_fast(out=rc, in_=pd)
                    o = out_pool.tile(
                        [128, NSL], BF16, tag="o", name=f"o_{b}_{h}_{half}"
                    )
                    nc.vector.tensor_mul(o, po[half], rc)
                    nc.scalar.dma_start(
                        out=out_ap[h, :, b, q0 : q0 + NSL], in_=o
                    )

            # ---- schedule: proj segments with scores pipelined in ----
            xs = [load_x(0), load_x(1)]
            proj(0, xs[0], [])
            for b in range(B):
                seg1 = 2 * b + 1
                if seg1 + 1 < NSEG:
                    xs.append(load_x(seg1 + 1))
                proj(seg1, xs[seg1],
                     scores_a_fillers(b, 0) + scores_a_fillers(b, 1))
                seg2 = seg1 + 1
                if seg2 < NSEG:
                    if seg2 + 1 < NSEG:
                        xs.append(load_x(seg2 + 1))
                    proj(seg2, xs[seg2],
                         scores_b_fillers(b, 0) + scores_b_fillers(b, 1))
                else:
                    for f in scores_b_fillers(b, 0) + scores_b_fillers(b, 1):
                        f()
                avones(b, 0)
                avones(b, 1)

    nc.compile()
    return nc


def _prep_shared(hidden_states):
    x2 = np.ascontiguousarray(hidden_states.reshape(T, D).T)  # [D, T]
    x_host = np.ascontiguousarray(
        x2.astype(BF).reshape(KC, 128, NSEG, NSL).transpose(1, 2, 0, 3)
    )  # [128, NSEG, KC, NSL]

    inv = 1.0 / (ROPE_BASE ** (np.arange(0, HD, 2, dtype=np.float64) / HD))
    f = np.outer(inv, np.arange(S, dtype=np.float64))  # [64, S]
    cosT = np.concatenate([np.cos(f), np.cos(f)], axis=0).astype(BF)
    # sign of rotate_half folded in: rot(q)[p] = -q[p+64] (p<64), q[p-64]
    sin_half = np.sin(f)
    sinS = np.concatenate([-sin_half, sin_half], axis=0).astype(BF)

    p = np.arange(128)[:, None]
    j = np.arange(128)[None, :]
    mask0 = np.ascontiguousarray((j >= p).astype(np.float32))
    return x_host, cosT, sinS, mask0


def _core_rows(c):
    h0, h1 = 2 * c, 2 * c + 1
    rows = []
    for part in range(3):  # q, k, v blocks
        for h in (h0, h1):
            base = h * 3 * HD + part * HD
            rows.extend(range(base, base + HD))
    return np.asarray(rows)


def _prep_core(w_qkv, b_qkv, c):
    rows = _core_rows(c)
    wT = np.ascontiguousarray(w_qkv[rows, :].T).astype(BF)  # [D, 768]
    wqk = np.ascontiguousarray(
        wT[:, : 4 * 128].reshape(KC, 128, 4, 128).transpose(1, 2, 0, 3)
    )  # [128, 4, KC, 128]
    wv = np.ascontiguousarray(
        wT[:, 4 * 128 :].reshape(KC, 128, 2 * HD).transpose(1, 0, 2)
    )  # [128, KC, 256]
    b_sel = b_qkv[rows]
    bqk = np.ascontiguousarray(b_sel[: 4 * 128].reshape(4, 128).T)  # [128, 4]
    bv = np.ascontiguousarray(
        np.broadcast_to(b_sel[4 * 128 :], (128, 2 * HD))
    )  # [128, 256]
    return wqk, wv, bqk, bv


def _make_in_maps(hidden_states, w_qkv, b_qkv):
    x_host, cosT, sinS, mask0 = _prep_shared(hidden_states)
    ones = np.ones((128, 128), BF)
    in_maps = []
    for c in range(NCORES):
        wqk, wv, bqk, bv = _prep_core(w_qkv, b_qkv, c)
        in_maps.append(
            {
                "x": x_host,
                "wqk": wqk,
                "wv": wv,
                "bqk": bqk,
                "bv": bv,
                "cosT": cosT,
                "sinS": sinS,
                "mask0": mask0,
                "ones": ones,
            }
        )
    return in_maps


def _assemble(results):
    outs = np.stack(
        [np.asarray(results[c]["out"]).astype(np.float32) for c in range(NCORES)]
    )
    # [NCORES, HPC, HD, B, S] -> [B, S, H*HD]
    return np.ascontiguousarray(
        outs.reshape(H, HD, B, S).transpose(2, 3, 0, 1).reshape(B, S, D)
    )


def run(hidden_states, w_qkv, b_qkv, trace=False):
    from concourse.bass_utils import run_bass_kernel_spmd

    if "nc" not in _CACHE:
        _CACHE["nc"] = _build_program()
    nc = _CACHE["nc"]
    in_maps = _make_in_maps(
        np.asarray(hidden_states, dtype=np.float32),
        np.asarray(w_qkv, dtype=np.float32),
        np.asarray(b_qkv, dtype=np.float32),
    )
    res = run_bass_kernel_spmd(
        nc, in_maps, core_ids=list(range(NCORES)), trace=trace
    )
    out = _assemble(res.results)
    return out, res


def kernel(hidden_states, w_qkv, b_qkv):
    trace = os.environ.get("KERNEL_TRACE", "0") == "1"
    out, _res = run(hidden_states, w_qkv, b_qkv, trace=trace)
    return out
